# revision 5
# baseline (speedup 1.0000x reference)
"""MHA kernel for Trainium2, 8-core tensor-parallel (2 heads per core).

Problem (hardcoded): x [2, 2048, 1024] fp32, Wq/Wk/Wv/Wo [1024, 1024],
bq/bk/bv/bo [1024], H=16 heads, DH=64.  out = MHA(x).

Sharding: heads are split 8 ways (2 heads = 128 proj columns per core).
Each core computes its heads' attention output and a partial output
projection (row-parallel Wo); the host sums the 8 partials and adds the
closed-form bias terms (bv @ Wo + bo).

v2 design (ScalarE-exp is the bottleneck engine at ~147us; everything
else is scheduled to hide under it):
  - scores S^T [128 k, 2h x 512 q] per ktile (two heads concurrently on
    PE row groups 0-63 / 64-127), exp on ScalarE per ktile (N=1024).
  - AV swapped: pt tile [128 k, 128 q] is the STATIONARY operand (full
    128-col array, FWL-eligible), V augmented with a ones column is the
    MOVING operand (N=65) -> out [128 q, 64 dh | denom] token-major with
    the softmax denominator accumulated for free in column 64.
  - normalize = DVE tensor_scalar divide by the per-partition denom.
  - transpose back to dh-major via one [128,128] identity matmul per
    q-subtile (both heads at once), then the usual row-parallel outproj.
  - QKV/V projections are interleaved into the attention phase's PE idle
    slots (PE has ~9us/combo spare under the 18.4us/combo exp pace).
  - partial outputs DMA'd out in bf16; host accumulates in fp32.
"""

import numpy as np
import ml_dtypes

D = 1024
T = 4096          # B*S tokens
S = 2048
B = 2
NH = 2            # heads per core
DH = 64
NCORES = 8
SCALE = 0.125     # 1/sqrt(DH)
NKT = S // 128    # 16 key tiles per batch
NQC = S // 512    # 4 query chunks per batch
NCK = T // 512    # 8 x^T column chunks
VSLOT = DH + 1    # 65: V columns + ones column

_CACHE = {}


def _build_nc(reps=1):
    import concourse.bacc as bacc
    import concourse.mybir as mybir
    import concourse.tile as tile

    dt = mybir.dt
    f32, bf16, i32 = dt.float32, dt.bfloat16, dt.int32

    nc = bacc.Bacc("TRN2", target_bir_lowering=False, debug=False,
                   num_devices=NCORES)

    xT = nc.dram_tensor("xT", [D, T], bf16, kind="ExternalInput")
    wq_d = nc.dram_tensor("wq", [D, 128], bf16, kind="ExternalInput")
    wk_d = nc.dram_tensor("wk", [D, 128], bf16, kind="ExternalInput")
    wv_d = nc.dram_tensor("wv", [D, 128], bf16, kind="ExternalInput")
    wo_d = nc.dram_tensor("wo", [128, D], bf16, kind="ExternalInput")
    bq_d = nc.dram_tensor("bq", [128, 1], f32, kind="ExternalInput")
    bk_d = nc.dram_tensor("bk", [128, 1], f32, kind="ExternalInput")
    outp = nc.dram_tensor("outp", [T, D], bf16, kind="ExternalOutput")

    with tile.TileContext(nc) as tc:
      for _rep in range(reps):
        with (
            tc.tile_pool(name="persist", bufs=1) as pp,
            tc.tile_pool(name="pt", bufs=2) as ptp,
            tc.tile_pool(name="ot", bufs=8) as otp,       # onorm_tok [128,128]
            tc.tile_pool(name="otT", bufs=6) as otTp,     # onormT [128,128]
            tc.tile_pool(name="dn", bufs=2) as dnp,       # denom sbuf [128,8]
            tc.tile_pool(name="outsb", bufs=3) as osp,
            tc.tile_pool(name="st_ps", bufs=2, space="PSUM") as stp,   # 4 banks
            tc.tile_pool(name="av_ps", bufs=1, space="PSUM") as avp,   # 2 banks
            tc.tile_pool(name="mm_ps", bufs=2, space="PSUM") as mmp,   # 2 banks
        ):
            # ---- constants / weights / x^T (one DMA per 512-col chunk,
            # ordered so K(b0,c0) can start as early as possible) ----
            wq = pp.tile([128, D], bf16, tag="wq")
            wk = pp.tile([128, D], bf16, tag="wk")
            wv = pp.tile([128, D], bf16, tag="wv")
            wo = pp.tile([128, D], bf16, tag="wo")
            bq = pp.tile([128, 1], f32, tag="bq")
            bk = pp.tile([128, 1], f32, tag="bk")
            xt_all = pp.tile([128, 8 * T], bf16, tag="xt")
            xt4 = xt_all.rearrange("p (d t) -> p d t", d=8)
            xsrc = xT.ap().rearrange("(d p) c -> p d c", p=128)

            def dma_w(w_sb, w_dr):
                nc.gpsimd.dma_start(
                    out=w_sb.rearrange("p (t c) -> p t c", c=128),
                    in_=w_dr.ap().rearrange("(t p) c -> p t c", p=128))

            def dma_cols(c0, c1):
                cs = slice(c0, c1)
                nc.sync.dma_start(out=xt4[:, :, cs], in_=xsrc[:, :, cs])

            def dma_chunk(nck):
                dma_cols(nck * 512, (nck + 1) * 512)

            # dummy exp on a zeroed scratch so the one ACT_TABLE_LOAD for
            # exp_and_others happens during the preamble, not on the
            # first combo's critical path.
            dummy = pp.tile([128, 1], f32, tag="dummy")
            nc.vector.memset(dummy[:, :], 0.0)
            nc.scalar.activation(dummy[:, :], dummy[:, :],
                                 mybir.ActivationFunctionType.Exp)

            dma_chunk(0)
            dma_w(wk, wk_d)
            nc.gpsimd.dma_start(out=bk[:, :], in_=bk_d.ap()[:, :])
            dma_w(wq, wq_d)
            nc.gpsimd.dma_start(out=bq[:, :], in_=bq_d.ap()[:, :])
            dma_chunk(1)
            dma_w(wv, wv_d)
            nc.gpsimd.dma_start(out=wo[:, :], in_=wo_d.ap()[:, :])
            for nck in range(2, NCK):
                dma_chunk(nck)
            ones1 = pp.tile([128, 1], f32, tag="ones1")
            nc.vector.memset(ones1[:, :], 1.0)

            # identity [128,128] bf16 for PE transposes: (f - p) == 0
            idx = pp.tile([128, 128], i32, tag="idx")
            nc.gpsimd.iota(idx[:, :], pattern=[[1, 128]], base=0,
                           channel_multiplier=-1)
            ident = pp.tile([128, 128], bf16, tag="ident")
            nc.vector.tensor_scalar(ident[:, :], idx[:, :], 0, None,
                                    op0=mybir.AluOpType.is_equal)

            # ---- persistent proj outputs ----
            qt = pp.tile([128, T], bf16, tag="qt")
            kt = pp.tile([128, T], bf16, tag="kt")
            vtm = []
            for b in range(B):
                v_sb = pp.tile([128, NH * NKT * VSLOT], bf16, tag=f"v{b}")
                v4 = v_sb.rearrange("p (h k c) -> p h k c", h=NH, k=NKT)
                nc.vector.memset(v4[:, :, :, DH:DH + 1], 1.0)
                vtm.append(v_sb)

            wq3 = wq.rearrange("p (t c) -> p t c", c=128)
            wk3 = wk.rearrange("p (t c) -> p t c", c=128)
            wv3 = wv.rearrange("p (t c) -> p t c", c=128)

            # single persistent AV accumulator (re-zeroed by each combo's
            # deferred norm chain, so the memset always follows the reads)
            av = avp.tile([128, NH * 4 * VSLOT], f32, tag="av")
            av4 = av.rearrange("p (h s c) -> p h s c", h=NH, s=4)
            nc.vector.memset(av[:, :], 0.0)

            # ---- filler work units (each: 8 chained MMs + 1 DVE evac) ----
            def emit_QU(b, qc, c0=0, c1=512):
                cs = slice(b * S + qc * 512 + c0, b * S + qc * 512 + c1)
                ps = mmp.tile([128, 512], f32, tag="mm",
                              name=f"qproj{b}_{qc}_{c0}")
                for d in range(8):
                    nc.tensor.matmul(ps[:, 0:c1 - c0], wq3[:, d, :],
                                     xt4[:, d, cs],
                                     start=(d == 0), stop=(d == 7))
                nc.vector.tensor_scalar_add(qt[:, cs], ps[:, 0:c1 - c0],
                                            bq[:, :])

            def emit_KU(b, c, c0=0, c1=512):
                cs = slice(b * S + c * 512 + c0, b * S + c * 512 + c1)
                ps = mmp.tile([128, 512], f32, tag="mm",
                              name=f"kproj{b}_{c}_{c0}")
                for d in range(8):
                    nc.tensor.matmul(ps[:, 0:c1 - c0], wk3[:, d, :],
                                     xt4[:, d, cs],
                                     start=(d == 0), stop=(d == 7))
                nc.vector.tensor_scalar_add(kt[:, cs], ps[:, 0:c1 - c0],
                                            bk[:, :])

            def emit_VU(b, k):
                tok0 = b * S + k * 128
                v4 = vtm[b].rearrange("p (h k c) -> p h k c", h=NH, k=NKT)
                ps = mmp.tile([128, 128], f32, tag="mm",
                              name=f"vproj{b}_{k}")
                for d in range(8):
                    nc.tensor.matmul(ps[:, :], xt4[:, d, tok0:tok0 + 128],
                                     wv3[:, d, :],
                                     start=(d == 0), stop=(d == 7))
                nc.vector.tensor_copy(
                    v4[:, :, k, 0:DH],
                    ps.rearrange("p (h c) -> p h c", h=NH)[:, :, :])

            # ---- micro-task factories: each task is <=1us of PE work,
            # popped one-per-ktile inside emit_combo so filler never forms
            # a multi-us lump between two score matmuls ----
            def KU_tasks(b, c):
                cs = slice(b * S + c * 512, b * S + c * 512 + 512)
                state = {}

                def ta():
                    ps = mmp.tile([128, 512], f32, tag="mm",
                                  name=f"kproj{b}_{c}a")
                    state["ps"] = ps
                    for d in range(4):
                        nc.tensor.matmul(ps[:, :], wk3[:, d, :],
                                         xt4[:, d, cs],
                                         start=(d == 0), stop=False)

                def tb():
                    ps = state["ps"]
                    for d in range(4, 8):
                        nc.tensor.matmul(ps[:, :], wk3[:, d, :],
                                         xt4[:, d, cs],
                                         start=False, stop=(d == 7))
                    nc.vector.tensor_scalar_add(kt[:, cs], ps[:, :],
                                                bk[:, :])
                return [ta, tb]

            def QU_tasks(b, qc):
                cs = slice(b * S + qc * 512, b * S + qc * 512 + 512)
                state = {}

                def ta():
                    ps = mmp.tile([128, 512], f32, tag="mm",
                                  name=f"qproj{b}_{qc}a")
                    state["ps"] = ps
                    for d in range(4):
                        nc.tensor.matmul(ps[:, :], wq3[:, d, :],
                                         xt4[:, d, cs],
                                         start=(d == 0), stop=False)

                def tb():
                    ps = state["ps"]
                    for d in range(4, 8):
                        nc.tensor.matmul(ps[:, :], wq3[:, d, :],
                                         xt4[:, d, cs],
                                         start=False, stop=(d == 7))
                    nc.vector.tensor_scalar_add(qt[:, cs], ps[:, :],
                                                bq[:, :])
                return [ta, tb]

            def VU_task(b, k):
                return lambda: emit_VU(b, k)

            def TO_tasks(m, tail=False):
                # transpose + outproj + out DMA for combo m, as 5 tasks
                # pipelined so each outproj pair's oT CAST has a full
                # ktile of slack: [T0], [P0,T1], [P1,T2], [P2,T3], [P3].
                # tail=True routes half the output casts to the (by then
                # idle) ScalarE so the final drain is not DVE-serial.
                b, qc = divmod(m, NQC)
                q0 = b * S + qc * 512
                state = {}

                def mk_T(s4):
                    def t():
                        ots = combo_ot[m]
                        tp = mmp.tile([128, 128], f32, tag="mm",
                                      name=f"tp{m}_{s4}")
                        nc.tensor.matmul(tp[:, :], ots[s4][:, :],
                                         ident[:, :], start=True, stop=True)
                        oT = otTp.tile([128, 128], bf16, tag="otT",
                                       name=f"otT{m}_{s4}")
                        nc.vector.tensor_copy(oT[:, :], tp[:, :])
                        state[s4] = oT
                    return t

                def mk_P(s4):
                    def t():
                        oT = state[s4]
                        for jc in range(2):
                            op = mmp.tile([128, 512], f32, tag="mm",
                                          name=f"op{m}_{s4}_{jc}")
                            nc.tensor.matmul(
                                op[:, :], oT[:, :],
                                wo[:, jc * 512:(jc + 1) * 512],
                                start=True, stop=True)
                            osb = osp.tile([128, 512], bf16, tag="outsb",
                                           name=f"osb{m}_{s4}_{jc}")
                            if tail and jc == 1:
                                nc.scalar.copy(osb[:, :], op[:, :])
                            else:
                                nc.vector.tensor_copy(osb[:, :], op[:, :])
                            r0 = q0 + s4 * 128
                            nc.sync.dma_start(
                                out=outp.ap()[r0:r0 + 128,
                                              jc * 512:(jc + 1) * 512],
                                in_=osb[:, :])
                        if s4 == 3:
                            combo_ot.pop(m)
                    return t

                def seq(*fs):
                    def t():
                        for f in fs:
                            f()
                    return t

                Ts = [mk_T(s) for s in range(4)]
                Ps = [mk_P(s) for s in range(4)]
                return [Ts[0], seq(Ps[0], Ts[1]), seq(Ps[1], Ts[2]),
                        seq(Ps[2], Ts[3]), Ps[3]]

            # ---- per-combo state ----
            combo_ot = {}    # m -> list of 4 onorm_tok tiles

            def emit_trans_out(m):
                # transpose (both heads at once) + outproj + out DMA for
                # combo m (emitted one combo later)
                b, qc = divmod(m, NQC)
                q0 = b * S + qc * 512
                ots = combo_ot.pop(m)
                oTs = []
                for s4 in range(4):
                    tp = mmp.tile([128, 128], f32, tag="mm",
                                  name=f"tp{m}_{s4}")
                    nc.tensor.matmul(tp[:, :], ots[s4][:, :], ident[:, :],
                                     start=True, stop=True)
                    oT = otTp.tile([128, 128], bf16, tag="otT",
                                   name=f"otT{m}_{s4}")
                    nc.vector.tensor_copy(oT[:, :], tp[:, :])
                    oTs.append(oT)
                for s4 in range(4):
                    oT = oTs[s4]
                    for jc in range(2):
                        op = mmp.tile([128, 512], f32, tag="mm",
                                      name=f"op{m}_{s4}_{jc}")
                        nc.tensor.matmul(
                            op[:, :], oT[:, :],
                            wo[:, jc * 512:(jc + 1) * 512],
                            start=True, stop=True)
                        osb = osp.tile([128, 512], bf16, tag="outsb",
                                       name=f"osb{m}_{s4}_{jc}")
                        nc.vector.tensor_copy(osb[:, :], op[:, :])
                        r0 = q0 + s4 * 128
                        nc.sync.dma_start(
                            out=outp.ap()[r0:r0 + 128,
                                          jc * 512:(jc + 1) * 512],
                            in_=osb[:, :])

            # ---- combo emission ----
            def emit_combo(m, tasks, vu_list=(), norm_prev=None,
                           norm_at=0, av_start=3, defer_last_av=False):
                """tasks: micro-task list, one popped per ktile; vu_list:
                V-proj units (b, kt) to spread one-per-ktile; norm_prev:
                the previous combo's deferred norm closure (emitted after
                ktile 1's exp so its Ln/Exp never stall the scalar
                queue).  AV matmuls for ktile k are interleaved right
                after the scores of ktile k+2 (exp(k) is done by then
                thanks to the st double-buffer pacing), so the PE never
                piles a serial AV block between combos."""
                b, qc = divmod(m, NQC)
                q0 = b * S + qc * 512
                v4 = vtm[b].rearrange("p (h k c) -> p h k c", h=NH, k=NKT)
                tasks = list(tasks)
                pt = ptp.tile([128, NH * NKT * 512], bf16, tag="pt",
                              name=f"pt{m}")
                pt3 = pt.rearrange("p (h k q) -> p h k q", h=NH, k=NKT)

                # AV swapped: pt stationary [128k,128q], V|1 moving (N=65).
                # The 8 (h,s4) accumulation groups interleave within shared
                # PSUM banks, so hardware zero-on-start (2KB region
                # granularity) would wipe neighbors: the tile is memset
                # (by the preamble / previous norm chain) and accumulated
                # with start=False throughout.
                def emit_av(kti):
                    for h in range(NH):
                        for s4 in range(4):
                            nc.tensor.matmul(
                                av4[:, h, s4, :],
                                pt3[:, h, kti, s4 * 128:(s4 + 1) * 128],
                                v4[:, h, kti, :],
                                start=False, stop=(kti == NKT - 1),
                                skip_group_check=True)

                for kti in range(NKT):
                    k0 = b * S + kti * 128
                    st = stp.tile([128, 1024], f32, tag="st",
                                  name=f"st{m}_{kti}")
                    # 2x2 quadrant tiling: each head's scores use only 64
                    # contraction rows, and each 64-key half uses only 64
                    # output partitions, so the four (h, khalf) matmuls
                    # occupy disjoint 64x64 PE quadrants and run
                    # concurrently (explicit tile_position).
                    for kh in range(2):
                        for h in range(NH):
                            hp = h * DH
                            nc.tensor.matmul(
                                st[kh * 64:(kh + 1) * 64,
                                   h * 512:(h + 1) * 512],
                                kt[hp:hp + DH,
                                   k0 + kh * 64:k0 + kh * 64 + 64],
                                qt[hp:hp + DH, q0:q0 + 512],
                                start=True, stop=True,
                                tile_position=(hp, kh * 64))
                    nc.scalar.activation(
                        pt3[:, :, kti, :], st[:, :],
                        mybir.ActivationFunctionType.Exp, scale=SCALE)
                    if kti == norm_at and norm_prev is not None:
                        norm_prev()
                    if kti < len(vu_list):
                        emit_VU(*vu_list[kti])
                    if kti >= av_start:
                        emit_av(kti - av_start)
                    if tasks:
                        tasks.pop(0)()
                for k in range(NKT - av_start,
                               NKT - (2 if defer_last_av else 0)):
                    emit_av(k)
                for t in tasks:
                    t()

                def norm():
                    # denoms -> SBUF, recip via the DVE hardware divide
                    # (keeps ScalarE free for the main exp stream), DVE
                    # mult, then re-zero the shared AV accumulator for the
                    # next combo (after all reads).
                    dn = dnp.tile([128, 2 * NH * 4], f32, tag="dn",
                                  name=f"dn{m}")
                    dn4 = dn.rearrange("p (g h s) -> p g h s", g=2, h=NH)
                    dn3 = dn4[:, 0, :, :]
                    rc3 = dn4[:, 1, :, :]
                    nc.vector.tensor_copy(dn3[:, :, :], av4[:, :, :, DH])
                    nc.vector.reciprocal(rc3[:, :, :], dn3[:, :, :])
                    ots = []
                    for s4 in range(4):
                        ot = otp.tile([128, 128], bf16, tag="ot",
                                      name=f"ot{m}_{s4}")
                        for h in range(NH):
                            nc.vector.tensor_scalar(
                                ot[:, h * DH:(h + 1) * DH],
                                av4[:, h, s4, 0:DH],
                                rc3[:, h, s4:s4 + 1], None,
                                op0=mybir.AluOpType.mult)
                        ots.append(ot)
                    combo_ot[m] = ots
                    nc.vector.memset(av[:, :], 0.0)
                return norm, emit_av

            # ---- schedule ----
            # PE warmup while the first x^T chunk DMA is in flight: ~16
            # dummy matmuls on the identity tile ramp the PE p-state so
            # the first projection chain runs at speed.
            for wi in range(40):
                wps = mmp.tile([128, 128], f32, tag="mm",
                               name=f"warm{wi}")
                nc.tensor.matmul(wps[:, :], ident[:, :], ident[:, :],
                                 start=True, stop=True)
            emit_KU(0, 0)
            emit_QU(0, 0)
            nopt = lambda: None
            n0, av0 = emit_combo(0,
                            KU_tasks(0, 1) + KU_tasks(0, 2)
                            + KU_tasks(0, 3) + QU_tasks(0, 1),
                            vu_list=[(0, k) for k in range(14)],
                            defer_last_av=True)
            n1, _ = emit_combo(1,
                            [VU_task(0, 14), VU_task(0, 15),
                             lambda: av0(14), lambda: av0(15)]
                            + QU_tasks(0, 2) + TO_tasks(0),
                            norm_prev=n0, norm_at=4, av_start=4)
            qu03 = QU_tasks(0, 3)
            ku10 = KU_tasks(1, 0)
            n2, _ = emit_combo(2,
                            [qu03[0], VU_task(1, 0), qu03[1],
                             VU_task(1, 1)] + TO_tasks(1)
                            + [ku10[0], VU_task(1, 2), ku10[1],
                               VU_task(1, 6), VU_task(1, 7)],
                            norm_prev=n1)
            qu10 = QU_tasks(1, 0)
            ku11 = KU_tasks(1, 1)
            ku12 = KU_tasks(1, 2)
            n3, _ = emit_combo(3,
                            [qu10[0], VU_task(1, 3), qu10[1],
                             VU_task(1, 4), ku11[0], VU_task(1, 5),
                             ku11[1], ku12[0], VU_task(1, 8), ku12[1]]
                            + TO_tasks(2),
                            norm_prev=n2)
            ku13 = KU_tasks(1, 3)
            qu11 = QU_tasks(1, 1)
            n4, _ = emit_combo(4,
                            [ku13[0], VU_task(1, 9), ku13[1],
                             VU_task(1, 10), qu11[0], VU_task(1, 11),
                             qu11[1], VU_task(1, 12), VU_task(1, 13),
                             VU_task(1, 14), VU_task(1, 15)],
                            norm_prev=n3)
            qu12 = QU_tasks(1, 2)
            to3 = TO_tasks(3)
            n5, _ = emit_combo(5,
                            [qu12[0], to3[0], qu12[1]] + to3[1:],
                            norm_prev=n4)
            qu13 = QU_tasks(1, 3)
            to4 = TO_tasks(4)
            n6, _ = emit_combo(6,
                            [qu13[0], to4[0], qu13[1]] + to4[1:]
                            + TO_tasks(5),
                            norm_prev=n5)
            n7, _ = emit_combo(7, [nopt, nopt] + TO_tasks(6),
                            norm_prev=n6)
            n7()
            for t in TO_tasks(7, tail=True):
                t()

    nc.compile()
    return nc


def _prep_inputs(x, Wq, bq, Wk, bk, Wv, bv, Wo, bo):
    bf16 = ml_dtypes.bfloat16
    xT = np.ascontiguousarray(
        np.asarray(x, dtype=np.float32).reshape(T, D).T).astype(bf16)
    in_maps = []
    for c in range(NCORES):
        cs = slice(c * 128, (c + 1) * 128)
        in_maps.append({
            "xT": xT,
            "wq": np.ascontiguousarray(Wq[:, cs]).astype(bf16),
            "wk": np.ascontiguousarray(Wk[:, cs]).astype(bf16),
            "wv": np.ascontiguousarray(Wv[:, cs]).astype(bf16),
            "wo": np.ascontiguousarray(Wo[cs, :]).astype(bf16),
            "bq": np.ascontiguousarray(bq[cs]).reshape(128, 1).astype(np.float32),
            "bk": np.ascontiguousarray(bk[cs]).reshape(128, 1).astype(np.float32),
        })
    return in_maps


def kernel(x, Wq, bq, Wk, bk, Wv, bv, Wo, bo, _trace=False, _results=None):
    from concourse.bass_utils import run_bass_kernel_spmd

    x = np.asarray(x); Wq = np.asarray(Wq); Wk = np.asarray(Wk)
    Wv = np.asarray(Wv); Wo = np.asarray(Wo)
    bq = np.asarray(bq); bk = np.asarray(bk); bv = np.asarray(bv)
    bo = np.asarray(bo)

    if "nc" not in _CACHE:
        _CACHE["nc"] = _build_nc()
    nc = _CACHE["nc"]

    in_maps = _prep_inputs(x, Wq, bq, Wk, bk, Wv, bv, Wo, bo)
    res = run_bass_kernel_spmd(
        nc, in_maps, core_ids=list(range(NCORES)), trace=_trace)
    if _results is not None:
        _results.append(res)

    acc = np.zeros((T, D), dtype=np.float32)
    for c in range(NCORES):
        acc += np.asarray(res.results[c]["outp"], dtype=np.float32)
    acc += bv.astype(np.float32) @ Wo.astype(np.float32) + bo.astype(np.float32)
    return acc.reshape(B, S, D)

